# revision 38
# baseline (speedup 1.0000x reference)
"""Causal GQA attention with RoPE for Trainium2, sharded over 8 NeuronCores.

Problem: x[4,1024,2048] @ wq/wk/wv -> RoPE -> causal GQA attention -> @ wo.
H=32 q-heads, KVH=8 kv-heads (GQA rep 4), D=64.

Sharding: core = 2*b + g  (b = batch 0..3, g = head-group 0..1).
Each core handles one batch and 16 q-heads / 4 kv-heads, computing a partial
output projection; the host sums the two head-group partials per batch.

Device layout:
  - inputs are bf16 (x transposed as xT [DIM, S], weights pre-permuted);
    bf16 matmuls run 1 PE cycle/moving row at any width and halve the input
    DMA traffic (the startup critical path).
  - q/k computed transposed (qT/kT [head_dim, S]) feeding scores directly;
    v natural [S, head_dim].
  - scores for both heads of a pair land in one 2-bank PSUM tile
    ([keys, 512*p + q]) so a single Exp covers both heads.
  - attn@v uses E [keys, q] chunks as the STATIONARY operand and v [keys, D]
    as the moving operand, producing attnout [q, D]: moving dim is D+1=65
    instead of the query width, halving PE cost vs streaming E.  A
    ones-column in v gives the softmax denominator per q-partition;
    normalization is a per-partition scale during the PSUM->SBUF move.
    PE transposes (vs identity) rebuild the [head_dim, S] layout for wo.
  - head-dim of q/k de-interleaved on the host so the RoPE pair-swap becomes
    a 32-partition block swap, folded into the sin-product tensor ops
    (reading PSUM at a shifted partition base); work split DVE/Pool.
  - softmax without max subtraction (scores bounded well inside fp32 range).
  - causal: masked key blocks skipped; diagonal blocks compute the reachable
    column range plus one 128-wide triangular bf16 mask multiply per head.
"""

import os

import numpy as np

import concourse.bacc as bacc
import concourse.bass as bass
import concourse.mybir as mybir
import concourse.tile as tile
from concourse.bass_utils import run_bass_kernel_spmd

B, S, DIM = 4, 1024, 2048
H, KVH, D = 32, 8, 64
HL = H // 2        # 16 q heads per core
KVL = KVH // 2     # 4 kv heads per core
QCOLS = HL * D     # 1024
KCOLS = KVL * D    # 256
NB = 512           # matmul moving-dim block (one PSUM bank of fp32)
P = 128

F32 = mybir.dt.float32
BF16 = mybir.dt.bfloat16


def build_program():
    nc = bacc.Bacc()

    xT = nc.dram_tensor("xT", [DIM, S], BF16, kind="ExternalInput")
    wq = nc.dram_tensor("wq", [DIM, QCOLS], BF16, kind="ExternalInput")
    wk = nc.dram_tensor("wk", [DIM, KCOLS], BF16, kind="ExternalInput")
    wv = nc.dram_tensor("wv", [DIM, KCOLS], BF16, kind="ExternalInput")
    wo = nc.dram_tensor("wo", [QCOLS, DIM], BF16, kind="ExternalInput")
    cosP = nc.dram_tensor("cosP", [P, S], F32, kind="ExternalInput")
    sinP = nc.dram_tensor("sinP", [P, S], F32, kind="ExternalInput")
    mask128 = nc.dram_tensor("mask128", [P, P], BF16, kind="ExternalInput")
    ident = nc.dram_tensor("ident", [P, P], BF16, kind="ExternalInput")
    outT = nc.dram_tensor("outT", [DIM, S], F32, kind="ExternalOutput")

    KC = DIM // P   # 16 contraction chunks
    Exp = mybir.ActivationFunctionType.Exp
    Copy = mybir.ActivationFunctionType.Copy

    with tile.TileContext(nc) as tc:
        from contextlib import ExitStack
        es = ExitStack()
        with es:
            const = es.enter_context(tc.tile_pool(name="const", bufs=1))
            kdupp = es.enter_context(tc.tile_pool(name="kdupp", bufs=1))
            vaugp = es.enter_context(tc.tile_pool(name="vaugp", bufs=1))
            aotp = es.enter_context(tc.tile_pool(name="aotp", bufs=1))
            wop = es.enter_context(tc.tile_pool(name="wop", bufs=1))
            outp = es.enter_context(tc.tile_pool(name="outp", bufs=3))
            psum = es.enter_context(
                tc.tile_pool(name="psum", bufs=2, space="PSUM"))

            # ---- constants ----
            cost = const.tile([P, S], F32, name="cost")
            sint = const.tile([P, S], F32, name="sint")
            maskt = const.tile([P, P], BF16, name="maskt")
            identt = const.tile([P, P], BF16, name="identt")

            # persistent activation tiles
            kdup = [kdupp.tile([P, S], BF16, name=f"kdup{i}")
                    for i in range(KVL)]
            vaug = [[vaugp.tile([P, D + 4], BF16, name=f"vaug{kv}_{ic}")
                     for ic in range(S // P)] for kv in range(KVL)]
            aot = [aotp.tile([P, S], BF16, name=f"aot{j}") for j in range(8)]

            inner = ExitStack()
            with inner:
                xtp = inner.enter_context(tc.tile_pool(name="xtp", bufs=1))
                wstp = inner.enter_context(tc.tile_pool(name="wstp", bufs=3))
                wvrp = inner.enter_context(tc.tile_pool(name="wvrp", bufs=1))
                qrtp = inner.enter_context(tc.tile_pool(name="qrtp", bufs=3))
                spool = inner.enter_context(tc.tile_pool(name="spool", bufs=2))
                epool = inner.enter_context(
                    tc.tile_pool(name="epool", bufs=14))
                rpool = inner.enter_context(tc.tile_pool(name="rpool", bufs=4))
                packp = inner.enter_context(
                    tc.tile_pool(name="packp", bufs=8))
                # PSUM (bank-granular): scores 2-bank pair tiles; the four
                # 65-col attn@v accumulators pack into one bank at 128-col
                # slots; the four 128x128 bf16 transposes pack into one bank.
                psum_sc = inner.enter_context(
                    tc.tile_pool(name="psum_sc", bufs=2, space="PSUM"))
                psum_oa = inner.enter_context(
                    tc.tile_pool(name="psum_oa", bufs=1, space="PSUM"))
                psum_tp = inner.enter_context(
                    tc.tile_pool(name="psum_tp", bufs=1, space="PSUM"))

                # K weights + first x group lead the DMA queue so the first
                # projection chain starts as early as possible.  Weight loads
                # keep >= 512B innermost runs (256+ bf16 columns): narrower
                # runs halve DMA bandwidth.
                wkall = wvrp.tile([P, KC * KCOLS], BF16, name="wkall")
                nc.sync.dma_start(
                    wkall[:].rearrange("p (c e) -> p c e", c=KC),
                    wk[:].rearrange("(c p) e -> p c e", p=P))

                xtg = []

                def load_xg(g):
                    tg = xtp.tile([P, 4 * S], BF16, name=f"xtg{g}")
                    nc.sync.dma_start(
                        tg[:].rearrange("p (c e) -> p c e", c=4),
                        xT[g * 4 * P:(g + 1) * 4 * P, :].rearrange(
                            "(c p) e -> p c e", p=P))
                    xtg.append(tg)

                load_xg(0)
                load_xg(1)
                load_xg(2)
                load_xg(3)
                xt = [xtg[c // 4][:, (c % 4) * S:(c % 4 + 1) * S]
                      for c in range(KC)]

                nc.sync.dma_start(cost[:], cosP[:])
                nc.sync.dma_start(sint[:], sinP[:])

                wvall = wvrp.tile([P, KC * KCOLS], BF16, name="wvall")
                nc.sync.dma_start(
                    wvall[:].rearrange("p (c e) -> p c e", c=KC),
                    wv[:].rearrange("(c p) e -> p c e", p=P))
                wvt = [wvall[:, c * KCOLS:(c + 1) * KCOLS] for c in range(KC)]

                # ones columns of vaug (runs while DMAs are in flight)
                for kv in range(KVL):
                    for ic in range(S // P):
                        nc.gpsimd.memset(vaug[kv][ic][:, D:D + 1], 1.0)

                def rope(ps, ib, dest_ap):
                    """psum [128, NB] f32 -> roped bf16 into dest_ap.

                    out = ps*cos + swap32(ps)*sin.  The 32-partition block
                    swap within each 64-row head is folded into the sin
                    multiplies by reading ps at a shifted partition base.
                    Work is split across DVE and Pool."""
                    sl = slice(ib * NB, (ib + 1) * NB)
                    tcos = spool.tile([P, NB], BF16, tag="tcos")
                    nc.vector.tensor_mul(tcos[:], ps[:], cost[:, sl])
                    tsin = spool.tile([P, NB], BF16, tag="tsin")
                    nc.vector.tensor_mul(
                        tsin[0:32, :], ps[32:64, :], sint[0:32, sl])
                    nc.gpsimd.tensor_mul(
                        tsin[32:64, :], ps[0:32, :], sint[32:64, sl])
                    nc.vector.tensor_mul(
                        tsin[64:96, :], ps[96:128, :], sint[64:96, sl])
                    nc.gpsimd.tensor_mul(
                        tsin[96:128, :], ps[64:96, :], sint[96:128, sl])
                    # all-bf16 SBUF add runs in DVE 2x mode
                    nc.vector.tensor_add(dest_ap, tcos[:], tsin[:])

                # ---- K projection + rope + duplicate ----
                # Chains are pairwise interleaved with group-rotated
                # contraction order so PE consumes x groups as they arrive
                # from DMA instead of all chains stalling on the last group.
                def gen_k(jk, ib, rot):
                    wkt = [wkall[:, c * KCOLS + jk * P:c * KCOLS + jk * P + P]
                           for c in range(KC)]
                    ps = psum.tile([P, NB], F32, tag="mm")
                    order = [(c + 4 * rot) % KC for c in range(KC)]
                    for idx, c in enumerate(order):
                        nc.tensor.matmul(
                            ps[:], wkt[c][:],
                            xt[c][:, ib * NB:(ib + 1) * NB],
                            start=(idx == 0), stop=(idx == KC - 1))
                        if idx % 4 == 3:
                            yield
                    kr = spool.tile([P, NB], BF16, tag="ropek")
                    rope(ps, ib, kr[:])
                    sl = slice(ib * NB, (ib + 1) * NB)
                    for half in range(2):  # kv head 2jk+half
                        src = kr[64 * half:64 * half + 64, :]
                        nc.vector.tensor_copy(
                            kdup[2 * jk + half][0:64, sl], src)
                        nc.vector.tensor_copy(
                            kdup[2 * jk + half][64:128, sl], src)
                    yield

                def gen_v():
                    """V projection (natural layout), bf16 copies on Pool."""
                    for ic in range(S // P):  # 8 key chunks
                        ps = psum.tile([P, KCOLS], F32, tag="mm")
                        for c in range(KC):
                            nc.tensor.matmul(
                                ps[:], xt[c][:, ic * P:(ic + 1) * P],
                                wvt[c][:],
                                start=(c == 0), stop=(c == KC - 1))
                        for kv in range(KVL):
                            nc.gpsimd.tensor_copy(vaug[kv][ic][:, 0:D],
                                                  ps[:, kv * D:(kv + 1) * D])
                        yield

                # Engines execute their instruction streams in emission
                # order, so PE groups from independent work (projection
                # chains, scores, attn@v, transposes, wo chains) are
                # round-robin interleaved via generators: PE never sits
                # behind a single cross-engine latency when other PE work
                # exists.
                def rr(*gens):
                    from collections import deque
                    q = deque(g for g in gens if g is not None)
                    while q:
                        g = q.popleft()
                        try:
                            next(g)
                            q.append(g)
                        except StopIteration:
                            pass

                def load_wq(t):
                    """Load q-chunk pair (2t, 2t+1) in one 512B-run DMA."""
                    wqg = wstp.tile([P, KC * 2 * P], BF16, tag="wchunk")
                    nc.sync.dma_start(
                        wqg[:].rearrange("p (c e) -> p c e", c=KC),
                        wq[:, t * 2 * P:(t + 1) * 2 * P].rearrange(
                            "(c p) e -> p c e", p=P))
                    return wqg

                wqgs = {0: load_wq(0)}
                nc.sync.dma_start(maskt[:], mask128[:])
                nc.sync.dma_start(identt[:], ident[:])

                # ---- K projection (x-arrival-paced pairs) ----
                rr(gen_k(0, 0, 0), gen_k(0, 1, 1))
                rr(gen_k(1, 0, 2), gen_k(1, 1, 3))

                # ---- Q projection + rope, interleaved with attention ----

                def load_wog(t):
                    """Load output-chunk pair (2t, 2t+1)."""
                    wog = wop.tile([P, 8 * 2 * P], BF16, tag=f"wot{t}",
                                   name=f"wog{t}")
                    nc.sync.dma_start(
                        wog[:].rearrange("p (c e) -> p c e", c=8),
                        wo[:, t * 2 * P:(t + 1) * 2 * P].rearrange(
                            "(c p) e -> p c e", p=P))
                    return wog

                def gen_qk_ib(jq, qr, wqg, ib):
                    half = (jq % 2) * P
                    ps = psum.tile([P, NB], F32, tag="mm")
                    for c in range(KC):
                        o = c * 2 * P + half
                        nc.tensor.matmul(
                            ps[:], wqg[:, o:o + P],
                            xt[c][:, ib * NB:(ib + 1) * NB],
                            start=(c == 0), stop=(c == KC - 1))
                        if c % 4 == 3:
                            yield
                    rope(ps, ib, qr[:, ib * NB:(ib + 1) * NB])
                    yield

                def gen_unit(jq, qr, qb):
                    """Scores+exp, then attn@v+normalize, then transpose for
                    both heads of q-chunk jq over query block qb.  Scores for
                    the head pair share a 2-bank PSUM tile (p=0 at [0:w],
                    p=1 at [512:512+w]) so one Exp covers both."""
                    kvh = jq // 2
                    nkj = 4 * (qb + 1)      # causal key chunks
                    Es = []
                    for kj in range(nkj):
                        c = kj - (nkj - 4)
                        off = P * c if c > 0 else 0
                        w = NB - off
                        sps = psum_sc.tile([P, 2 * NB], F32, tag="sc")
                        for p in range(2):
                            hsl = slice(64 * p, 64 * p + 64)
                            nc.tensor.matmul(
                                sps[:, p * NB:p * NB + w],
                                kdup[kvh][hsl, kj * P:(kj + 1) * P],
                                qr[hsl, qb * NB + off:(qb + 1) * NB],
                                start=True, stop=True)
                        E = epool.tile([P, 2 * NB], BF16, tag="E")
                        if w <= P:
                            # merging would be mostly pad; exp each head
                            nc.scalar.activation(E[:, 0:w], sps[:, 0:w], Exp)
                            nc.scalar.activation(
                                E[:, NB:NB + w], sps[:, NB:NB + w], Exp)
                        else:
                            nc.scalar.activation(
                                E[:, 0:NB + w], sps[:, 0:NB + w], Exp)
                        if c >= 0:
                            # triangular mask on each head's leading 128 cols
                            nc.vector.tensor_mul(
                                E[:, 0:P], E[:, 0:P], maskt[:])
                            nc.vector.tensor_mul(
                                E[:, NB:NB + P], E[:, NB:NB + P], maskt[:])
                        Es.append((E, off))
                        yield
                    # NOTE: no yields while an oa/tp tile (shallow shared
                    # PSUM pools) is held: a yield there lets another unit
                    # emit PE ops that wait on our not-yet-emitted releases,
                    # deadlocking the in-order PE stream.
                    pks = [packp.tile([P, P], BF16, tag="pk", name=f"pk{i}")
                           for i in range(4)]
                    for p in range(2):
                        oab = psum_oa.tile([P, NB], F32, tag="oa")
                        for qc in range(4):
                            kmax = 4 * qb + qc   # last causal key chunk
                            oa = oab[:, qc * P:qc * P + D + 1]
                            for kc in range(kmax + 1):
                                E, off = Es[kc]
                                colo = p * NB + qc * P - off
                                nc.tensor.matmul(
                                    oa,
                                    E[:, colo:colo + P],
                                    vaug[kvh][kc][:, 0:D + 1],
                                    start=(kc == 0), stop=(kc == kmax))
                            rec = rpool.tile([P, 1], F32, tag="rec")
                            nc.vector.reciprocal(
                                rec[:], oab[:, qc * P + D:qc * P + D + 1])
                            if p == 0:
                                nc.gpsimd.tensor_scalar_mul(
                                    pks[qc][:, 0:D],
                                    oab[:, qc * P:qc * P + D], rec[:])
                            else:
                                nc.vector.tensor_scalar_mul(
                                    pks[qc][:, D:2 * D],
                                    oab[:, qc * P:qc * P + D], rec[:])
                            if qc == 1:
                                yield
                        yield
                    tpb = psum_tp.tile([P, 4 * P], BF16, tag="tp")
                    for qc in range(4):
                        nc.tensor.transpose(
                            tpb[:, qc * P:(qc + 1) * P], pks[qc][:],
                            identt[:])
                        qsl = slice(qb * NB + qc * P, qb * NB + (qc + 1) * P)
                        nc.gpsimd.tensor_copy(
                            aot[jq][:, qsl], tpb[:, qc * P:(qc + 1) * P])
                    yield

                def gen_wo(ib, lo, hi):
                    """wo chains for output chunks [lo, hi) over query half
                    ib.  Only needs aot[*][:, ib*512:(ib+1)*512]."""
                    for n in range(lo, hi):
                        wog = wogs[n // 2]
                        half = (n % 2) * P
                        # split the very last piece finer for a fast drain
                        pieces = 2 if (ib == 1 and n == DIM // P - 1) else 1
                        pw = NB // pieces
                        for pc in range(pieces):
                            fps = psum.tile([P, NB], F32, tag="mm")
                            o0 = ib * NB + pc * pw
                            for hd in range(8):
                                o = hd * 2 * P + half
                                nc.tensor.matmul(
                                    fps[:, 0:pw],
                                    wog[:, o:o + P],
                                    aot[hd][:, o0:o0 + pw],
                                    start=(hd == 0), stop=(hd == 7))
                            osb = outp.tile([P, NB], F32, tag="osb")
                            nc.gpsimd.tensor_copy(osb[:, 0:pw], fps[:, 0:pw])
                            nc.sync.dma_start(
                                outT[n * P:(n + 1) * P, o0:o0 + pw],
                                osb[:, 0:pw])
                            yield

                # stage pairing: each rr() couples one projection chain with
                # one attention unit (which lags its rope by one stage), and
                # the V projection with the first Q chain; the final units
                # overlap the first wo half.
                wogs = {}
                qrs = {}
                for jq in range(QCOLS // P):  # 8 q chunks
                    if jq % 2 == 1 and (jq + 1) // 2 < QCOLS // (2 * P):
                        wqgs[(jq + 1) // 2] = load_wq((jq + 1) // 2)
                    qr = qrtp.tile([P, S], BF16, tag="qr")
                    qrs[jq] = qr
                    wqg = wqgs[jq // 2]
                    if jq == 0:
                        rr(gen_v(), gen_qk_ib(jq, qr, wqg, 0))
                    else:
                        rr(gen_qk_ib(jq, qr, wqg, 0),
                           gen_unit(jq - 1, qrs[jq - 1], 1))
                    wogs[jq] = load_wog(jq)
                    rr(gen_qk_ib(jq, qr, wqg, 1),
                       gen_unit(jq, qrs[jq], 0))
                # last qb1 unit overlapped with the first wo half
                rr(gen_unit(7, qrs[7], 1), gen_wo(0, 0, DIM // P))
                rr(gen_wo(1, 0, DIM // P))

    nc.compile()
    return nc


def host_inputs(x, freqs_cos, freqs_sin, wq, wk, wv, wo):
    """Build the 8 per-core input maps."""
    bf16 = mybir.dt.np(BF16)
    x = np.asarray(x, np.float32)
    cos = np.asarray(freqs_cos, np.float32)
    sin = np.asarray(freqs_sin, np.float32)
    wq = np.asarray(wq, np.float32)
    wk = np.asarray(wk, np.float32)
    wv = np.asarray(wv, np.float32)
    wo = np.asarray(wo, np.float32)

    perm = np.concatenate([np.arange(0, D, 2), np.arange(1, D, 2)])

    # cos/sin tiles in de-interleaved layout, [128, S] (two 64-row heads)
    cc = cos.T  # [32, S]
    ss = sin.T
    cos64 = np.concatenate([cc, cc], 0)
    sin64 = np.concatenate([-ss, ss], 0)
    cosP = np.ascontiguousarray(np.concatenate([cos64, cos64], 0))
    sinP = np.ascontiguousarray(np.concatenate([sin64, sin64], 0))

    # triangular causal mask for a diagonal 128x128 block: keys j, queries i
    j = np.arange(P)[:, None]
    i = np.arange(P)[None, :]
    mask128 = np.ascontiguousarray((j <= i).astype(bf16))
    ident = np.ascontiguousarray(np.eye(P, dtype=np.float32).astype(bf16))

    scale = np.float32(1.0 / np.sqrt(D))
    in_maps = []
    for core in range(8):
        b, g = core // 2, core % 2
        wq_g = wq[:, g * QCOLS:(g + 1) * QCOLS].reshape(DIM, HL, D)
        wq_g = (wq_g[:, :, perm] * scale).reshape(DIM, QCOLS)
        wk_g = wk[:, g * KCOLS:(g + 1) * KCOLS].reshape(DIM, KVL, D)
        wk_g = wk_g[:, :, perm].reshape(DIM, KCOLS)
        in_maps.append({
            "xT": np.ascontiguousarray(x[b].T.astype(bf16)),
            "wq": np.ascontiguousarray(wq_g.astype(bf16)),
            "wk": np.ascontiguousarray(wk_g.astype(bf16)),
            "wv": np.ascontiguousarray(
                wv[:, g * KCOLS:(g + 1) * KCOLS].astype(bf16)),
            "wo": np.ascontiguousarray(
                wo[g * QCOLS:(g + 1) * QCOLS, :].astype(bf16)),
            "cosP": cosP,
            "sinP": sinP,
            "mask128": mask128,
            "ident": ident,
        })
    return in_maps


_PROGRAM = None


def kernel(x, freqs_cos, freqs_sin, wq, wk, wv, wo):
    global _PROGRAM
    if _PROGRAM is None:
        _PROGRAM = build_program()
    nc = _PROGRAM
    in_maps = host_inputs(x, freqs_cos, freqs_sin, wq, wk, wv, wo)
    trace = os.environ.get("KERNEL_TRACE", "") == "1"
    if not trace:
        # the axon build here lacks the NTFF profile hook; make sure an
        # ambient BASS_TRACE can't route us into that (crashing) path
        os.environ["BASS_NEVER_TRACE"] = "1"
    res = run_bass_kernel_spmd(nc, in_maps, core_ids=list(range(8)),
                               trace=trace)
    if trace and res.exec_time_ns is not None:
        print(f"HW exec time: {res.exec_time_ns} ns")
        print(f"mean exec time: {res.mean_exec_time_ns} ns")
        if res.instructions_and_trace is not None:
            print("trace:", res.instructions_and_trace[1])
    out = np.zeros((B, S, DIM), np.float32)
    for core in range(8):
        b = core // 2
        out[b] += res.results[core]["outT"].T
    return out
